# revision 35
# baseline (speedup 1.0000x reference)
"""BilinearRelationNet Trainium2 kernel (8 NeuronCores, data-parallel over batch).

Layout strategy: feature-on-partitions, batch-on-free-dim ("feature-major").
  phase 1: stream x (fp32) -> cast fp16 -> xbar-transpose -> h_pre = x@W1 in PSUM
           (fp32), bn_stats per chunk, store h_pre as fp16 in SBUF.
  AllReduce: per-tower/per-h-chunk (mean, E[h^2]) across the 8 cores (4 KB).
  phase 2: BN affine+relu -> @W2+relu -> elementwise combine -> @W3ext
           (W3 extended with ones columns so dot/n1/n2 row-sums ride along in
           the same PSUM tile) -> relu -> @W4 -> stage per-row scalars.
  phase 3: gather per-row scalars into [128, R/128] tiles, cosine+sigmoid
           finalization, DMA out.

b1 is dropped: BatchNorm subtracts the batch mean of (x@W1 + b1), so b1
cancels exactly.
"""

import sys

sys.path.insert(0, "/opt/trn_rl_repo")

import numpy as np
import concourse.bass as bass
import concourse.bacc as bacc
import concourse.tile as tile
import concourse.mybir as mybir
from concourse import bass_utils

F32 = mybir.dt.float32
F16 = mybir.dt.float16
AF = mybir.ActivationFunctionType
ALU = mybir.AluOpType

N_CORES = 8
D = 512
H = 256
BN_EPS = 1e-5


def build_nc(n_chunks: int):
    """One SPMD program; each core handles R = n_chunks*512 rows of both x1/x2."""
    R = n_chunks * 512
    nc = bacc.Bacc("TRN2", target_bir_lowering=False, debug=False, num_devices=N_CORES)

    x_dram = [
        nc.dram_tensor("x1", [R, D], F32, kind="ExternalInput"),
        nc.dram_tensor("x2", [R, D], F32, kind="ExternalInput"),
    ]
    w1_d = nc.dram_tensor("w1p", [4, 128, H], F16, kind="ExternalInput")
    w2_d = nc.dram_tensor("w2p", [2, 128, 128], F16, kind="ExternalInput")
    w3_d = nc.dram_tensor("w3e", [5, 128, 67], F16, kind="ExternalInput")
    w4_d = nc.dram_tensor("w4p", [64, 1], F16, kind="ExternalInput")
    b2_d = nc.dram_tensor("b2c", [128, 1], F32, kind="ExternalInput")
    b3_d = nc.dram_tensor("b3c", [64, 1], F32, kind="ExternalInput")
    b4_d = nc.dram_tensor("b4c", [128, 1], F32, kind="ExternalInput")
    gamma_d = nc.dram_tensor("gamma2", [128, 2], F32, kind="ExternalInput")
    betabn_d = nc.dram_tensor("betabn2", [128, 2], F32, kind="ExternalInput")
    alpha_d = nc.dram_tensor("alphab", [128, 1], F32, kind="ExternalInput")
    beta_d = nc.dram_tensor("betab", [128, 1], F32, kind="ExternalInput")
    iden_d = nc.dram_tensor("iden", [128, 128], F16, kind="ExternalInput")
    b2r_d = nc.dram_tensor("b2r", [1, 128], F16, kind="ExternalInput")
    b3r_d = nc.dram_tensor("b3r", [1, 67], F16, kind="ExternalInput")
    out_d = nc.dram_tensor("out", [R], F32, kind="ExternalOutput")

    with tile.TileContext(nc) as tc:
        with (
            tc.tile_pool(name="const", bufs=1) as cpool,
            tc.tile_pool(name="persist", bufs=1) as hpool,
            tc.tile_pool(name="dram", bufs=1, space="DRAM") as dpool,
        ):
            # ---- constants to SBUF ----
            w1s = cpool.tile([128, 4 * H], F16, tag="w1s")
            for dc in range(4):
                nc.sync.dma_start(w1s[:, dc * H : (dc + 1) * H], w1_d[dc])
            w2s = cpool.tile([128, 2 * 128], F16, tag="w2s")
            for k in range(2):
                nc.sync.dma_start(w2s[:, k * 128 : (k + 1) * 128], w2_d[k])
            w3s = cpool.tile([128, 5 * 67], F16, tag="w3s")
            for k in range(5):
                nc.sync.dma_start(w3s[:, k * 67 : (k + 1) * 67], w3_d[k])
            w4s = cpool.tile([64, 1], F16, tag="w4s")
            nc.sync.dma_start(w4s[:], w4_d[:])
            b2s = cpool.tile([128, 1], F32, tag="b2s")
            nc.sync.dma_start(b2s[:], b2_d[:])
            b3s = cpool.tile([64, 1], F32, tag="b3s")
            nc.sync.dma_start(b3s[:], b3_d[:])
            b4s = cpool.tile([128, 1], F32, tag="b4s")
            nc.sync.dma_start(b4s[:], b4_d[:])
            gammas = cpool.tile([128, 2], F32, tag="gammas")
            nc.sync.dma_start(gammas[:], gamma_d[:])
            betabns = cpool.tile([128, 2], F32, tag="betabns")
            nc.sync.dma_start(betabns[:], betabn_d[:])
            alphas = cpool.tile([128, 1], F32, tag="alphas")
            nc.sync.dma_start(alphas[:], alpha_d[:])
            betas = cpool.tile([128, 1], F32, tag="betas")
            nc.sync.dma_start(betas[:], beta_d[:])
            idens = cpool.tile([128, 128], F16, tag="idens")
            nc.sync.dma_start(idens[:], iden_d[:])
            b2rs = cpool.tile([1, 128], F16, tag="b2rs")
            nc.sync.dma_start(b2rs[:], b2r_d[:])
            b3rs = cpool.tile([1, 67], F16, tag="b3rs")
            nc.sync.dma_start(b3rs[:], b3r_d[:])
            ones_row = cpool.tile([1, 512], F16, tag="ones_row")
            nc.vector.memset(ones_row[:], 1.0)

            # ---- persistent buffers ----
            # h_pre fp16, per (tower, h-chunk): [128, R]
            # per tower, chunk-interleaved: cols c*1024 + m*512 + j
            hp = [
                hpool.tile([128, 2 * R], F16, tag=f"hp{t}", name=f"hp{t}")
                for t in range(2)
            ]
            # bn_stats accumulators per stat-set (t,m): [128, 6*n_chunks]
            sbstats = [
                hpool.tile([128, 6 * n_chunks], F32, tag=f"bst{s}", name=f"bst{s}") for s in range(4)
            ]
            # per-row scalar gather targets: dot, n1, n2, slearn
            ncols = R // 128
            tq = [hpool.tile([128, ncols], F32, tag=f"tq{q}", name=f"tq{q}") for q in range(4)]
            # BN scale/shift per stat-set
            gsh = hpool.tile([128, 8], F32, tag="gsh")

            # ================= phase 1 =================
            # fp32->fp16 cast via SWDGE casting DMA (DRAM->SBUF), transpose on
            # TensorE (fp16, PSUM out), evacuate PSUM->SBUF in [128,512] strips.
            # Towers are processed sequentially so each tower's stats AllReduce
            # overlaps with the next tower's / phase 2's compute.
            stw = hpool.tile([128, 16], F32, tag="stw")  # scratch
            st2 = hpool.tile([128, 16], F32, tag="st2")  # scratch 2
            arin = hpool.tile([128, 8], F32, tag="arin")
            arout = hpool.tile([128, 8], F32, tag="arout")
            with (
                tc.tile_pool(name="p1sb", bufs=3) as p1,
                tc.tile_pool(name="p1ps", bufs=2, space="PSUM") as pp1,
            ):
                for t in range(2):
                    for c in range(n_chunks):
                        rows = slice(c * 512, (c + 1) * 512)
                        xfs = p1.tile([128, 4 * 512], F16, tag="xfs")
                        nc.gpsimd.dma_start(
                            xfs.rearrange("p (nb d) -> p nb d", nb=4),
                            x_dram[t][rows, :].rearrange("(nb p) d -> p nb d", p=128),
                        )
                        xT = p1.tile([128, 4 * 512], F16, tag="xT")
                        for half in range(2):
                            # one fp16 PSUM bank holds 8 transposed blocks
                            ptr = pp1.tile([128, 1024], F16, tag="ptr", bufs=3)
                            for dci in range(2):
                                dc = half * 2 + dci
                                for nb in range(4):
                                    nc.tensor.transpose(
                                        ptr[:, dci * 512 + nb * 128 : dci * 512 + (nb + 1) * 128],
                                        xfs[:, nb * 512 + dc * 128 : nb * 512 + (dc + 1) * 128],
                                        idens[:],
                                    )
                            nc.vector.tensor_copy(
                                xT[:, half * 1024 : (half + 1) * 1024], ptr[:]
                            )
                        ps = pp1.tile([128, 1024], F32, tag="ps", bufs=2)
                        for m in range(2):
                            pm = ps[:, m * 512 : (m + 1) * 512]
                            for dc in range(4):
                                nc.tensor.matmul(
                                    pm,
                                    w1s[:, dc * H + m * 128 : dc * H + (m + 1) * 128],
                                    xT[:, dc * 512 : (dc + 1) * 512],
                                    start=(dc == 0),
                                    stop=(dc == 3),
                                )
                            s = t * 2 + m
                            nc.vector.bn_stats(
                                sbstats[s][:, c * 6 : (c + 1) * 6], pm
                            )
                        nc.scalar.activation(
                            hp[t][:, c * 1024 : (c + 1) * 1024], ps[:], AF.Copy
                        )

                    # ---- tower-t stats + AllReduce (overlaps later work) ----
                    for m in range(2):
                        s = t * 2 + m
                        aggr = stw[:, s * 2 : s * 2 + 2]
                        nc.vector.bn_aggr(aggr, sbstats[s][:])
                        # arin[2m] = mean ; arin[2m+1] = E[h^2] = var + mean^2
                        nc.vector.tensor_copy(
                            arin[:, 4 * t + 2 * m : 4 * t + 2 * m + 1], aggr[:, 0:1]
                        )
                        msq = stw[:, 8 + s : 9 + s]
                        nc.vector.tensor_tensor(msq, aggr[:, 0:1], aggr[:, 0:1], ALU.mult)
                        nc.vector.tensor_tensor(
                            arin[:, 4 * t + 2 * m + 1 : 4 * t + 2 * m + 2],
                            aggr[:, 1:2], msq, ALU.add,
                        )
                # The Tile scheduler effectively barriers the chip around each
                # collective, so use ONE small AllGather (lower floor than
                # AllReduce) and do the 8-way sum locally on DVE.
                bnc_in = dpool.tile([128, 8], F32, name="bnc_in")
                bnc_out = dpool.tile(
                    [128 * N_CORES, 8], F32, addr_space="Shared", name="bnc_out"
                )
                nc.sync.dma_start(bnc_in[:], arin[:])
                nc.gpsimd.collective_compute(
                    "AllGather",
                    ALU.bypass,
                    ins=[bnc_in.opt()],
                    outs=[bnc_out.opt()],
                    replica_groups=[list(range(N_CORES))],
                )
                # gather all ranks' stats into [128, 8 ranks * 8 cols]
                agout = hpool.tile([128, 64], F32, tag="agout")
                nc.sync.dma_start(
                    agout.rearrange("p (r j) -> p r j", r=N_CORES),
                    bnc_out.rearrange("(r p) j -> p r j", p=128),
                )
                # tree-sum the 8 rank blocks -> arout [128, 8]
                tsum = hpool.tile([128, 32], F32, tag="tsum")
                for i in range(4):
                    nc.vector.tensor_tensor(
                        tsum[:, 8 * i : 8 * i + 8],
                        agout[:, 16 * i : 16 * i + 8],
                        agout[:, 16 * i + 8 : 16 * i + 16],
                        ALU.add,
                    )
                for i in range(2):
                    nc.vector.tensor_tensor(
                        tsum[:, 8 * i : 8 * i + 8],
                        tsum[:, 16 * i : 16 * i + 8],
                        tsum[:, 16 * i + 8 : 16 * i + 16],
                        ALU.add,
                    )
                nc.vector.tensor_tensor(
                    arout[:], tsum[:, 0:8], tsum[:, 8:16], ALU.add
                )

                # vectorized across the 4 stat-sets via strided column APs:
                # arout cols [0,2,4,6]=sum(mean_l), [1,3,5,7]=sum(Eh2_l)
                gm4 = st2[:, 0:4]
                nc.vector.tensor_scalar(
                    gm4, arout[:, 0:8:2], 1.0 / N_CORES, None, ALU.mult
                )
                ge4 = st2[:, 4:8]
                nc.vector.tensor_scalar(
                    ge4, arout[:, 1:8:2], 1.0 / N_CORES, None, ALU.mult
                )
                gm2_4 = st2[:, 8:12]
                nc.vector.tensor_tensor(gm2_4, gm4, gm4, ALU.mult)
                var4 = st2[:, 12:16]
                nc.vector.tensor_tensor(var4, ge4, gm2_4, ALU.subtract)
                vare4 = stw[:, 0:4]
                nc.vector.tensor_scalar(vare4, var4, float(BN_EPS), None, ALU.add)
                std4 = stw[:, 4:8]
                nc.scalar.activation(std4, vare4, AF.Sqrt)
                istd4 = stw[:, 8:12]
                nc.vector.reciprocal(istd4, std4)
                # gamma/beta col m for set s=(t,m): order gamma cols [0,1,0,1]
                gam4 = stw[:, 12:16]
                nc.vector.tensor_copy(gam4[:, 0:2], gammas[:])
                nc.vector.tensor_copy(gam4[:, 2:4], gammas[:])
                bet4 = st2[:, 0:4]  # gm4 no longer needed after gmg
                gmg4 = st2[:, 4:8]  # reuse ge4 slot after use
                nc.vector.tensor_tensor(
                    gsh[:, 0:8:2], istd4, gam4, ALU.mult
                )
                nc.vector.tensor_tensor(gmg4, gm4, gsh[:, 0:8:2], ALU.mult)
                nc.vector.tensor_copy(bet4[:, 0:2], betabns[:])
                nc.vector.tensor_copy(bet4[:, 2:4], betabns[:])
                nc.vector.tensor_tensor(
                    gsh[:, 1:8:2], bet4, gmg4, ALU.subtract
                )

            # ================= phase 2 =================
            # Process CW=1024 column blocks (2 chunks) to amortize the fixed
            # per-op engine overheads (ACT ~224cyc/op). Matmuls still target
            # one PSUM bank (N=512) per instruction.
            CW = 1024 if n_chunks % 2 == 0 else 512
            NH = CW // 512  # matmul halves per block
            with (
                tc.tile_pool(name="p2sb", bufs=3) as p2,
                tc.tile_pool(name="p2ps", bufs=1, space="PSUM") as pp2,
            ):
                for c in range(R // CW):
                    cols = slice(c * CW, (c + 1) * CW)
                    h = []
                    for t in range(2):
                        hn = []
                        for m in range(2):
                            s = t * 2 + m
                            hpv = hp[t].rearrange("p (c mm j) -> p c mm j", mm=2, j=512)
                            nck = CW // 512
                            src = hpv[:, c * nck : (c + 1) * nck, m, :]
                            dst = p2.tile([128, CW], F16, tag=f"hn{t}{m}")
                            dst3 = dst.rearrange("p (k j) -> p k j", j=512)
                            if s < 2:
                                nc.scalar.activation(
                                    dst3, src, AF.Relu,
                                    scale=gsh[:, 2 * s : 2 * s + 1],
                                    bias=gsh[:, 2 * s + 1 : 2 * s + 2],
                                )
                            else:
                                tmp = p2.tile([128, CW], F16, tag=f"hntmp{t}{m}")
                                tmp3 = tmp.rearrange("p (k j) -> p k j", j=512)
                                nc.vector.tensor_scalar(
                                    tmp3, src,
                                    gsh[:, 2 * s : 2 * s + 1],
                                    gsh[:, 2 * s + 1 : 2 * s + 2],
                                    ALU.mult, ALU.add,
                                )
                                nc.vector.tensor_scalar(
                                    dst, tmp, 0.0, None, ALU.max
                                )
                            hn.append(dst)
                        pw = pp2.tile([128, CW], F32, tag=f"pw{t}", bufs=1)
                        for hf in range(NH):
                            hs = slice(hf * 512, (hf + 1) * 512)
                            nc.tensor.matmul(
                                pw[:, hs], w2s[:, 0:128], hn[0][:, hs],
                                start=True, stop=False,
                            )
                            nc.tensor.matmul(
                                pw[:, hs], w2s[:, 128:256], hn[1][:, hs],
                                start=False, stop=False,
                            )
                            nc.tensor.matmul(
                                pw[:, hs], b2rs[:], ones_row[:],
                                start=False, stop=True,
                            )
                        ht = p2.tile([128, CW], F16, tag=f"h{t}")
                        if t == 0:
                            nc.scalar.activation(ht, pw[:], AF.Relu)
                        else:
                            nc.vector.tensor_scalar(ht, pw[:], 0.0, None, ALU.max)
                        h.append(ht)

                    p_t = p2.tile([128, CW], F16, tag="p_t")
                    nc.vector.tensor_tensor(p_t[:], h[0][:], h[1][:], ALU.mult)
                    dd = p2.tile([128, CW], F16, tag="dd")
                    nc.vector.tensor_tensor(dd[:], h[0][:], h[1][:], ALU.subtract)
                    q_t = p2.tile([128, CW], F16, tag="q_t")
                    nc.scalar.activation(q_t[:], dd[:], AF.Abs)
                    r_t = p2.tile([128, CW], F16, tag="r_t")
                    nc.vector.tensor_tensor(r_t[:], h[0][:], h[1][:], ALU.add)
                    s1_t = p2.tile([128, CW], F16, tag="s1_t")
                    nc.scalar.activation(s1_t[:], h[0][:], AF.Square)
                    s2_t = p2.tile([128, CW], F16, tag="s2_t")
                    nc.gpsimd.tensor_tensor(s2_t[:], h[1][:], h[1][:], ALU.mult)

                    pw3 = pp2.tile([128, CW], F32, tag="pw3", bufs=1)
                    rhs5 = [p_t, q_t, r_t, s1_t, s2_t]
                    for hf in range(NH):
                        hs = slice(hf * 512, (hf + 1) * 512)
                        for k in range(5):
                            nc.tensor.matmul(
                                pw3[0:67, hs],
                                w3s[:, k * 67 : (k + 1) * 67],
                                rhs5[k][:, hs],
                                start=(k == 0),
                                stop=False,
                            )
                        nc.tensor.matmul(
                            pw3[0:67, hs], b3rs[:], ones_row[:],
                            start=False, stop=True,
                        )
                    r64 = p2.tile([64, CW], F16, tag="r64")
                    nc.vector.tensor_scalar(r64[:], pw3[0:64, :], 0.0, None, ALU.max)
                    stage = p2.tile([33, CW], F32, tag="stage")
                    nc.scalar.activation(stage[0:3, :], pw3[64:67, :], AF.Copy)
                    pw4 = pp2.tile([1, CW], F32, tag="pw4", bufs=1)
                    for hf in range(NH):
                        hs = slice(hf * 512, (hf + 1) * 512)
                        nc.tensor.matmul(
                            pw4[:, hs], w4s[:], r64[:, hs], start=True, stop=True
                        )
                    nc.vector.tensor_copy(stage[32:33, :], pw4[:])
                    ppc = CW // ncols  # partition rows of tq covered per block
                    for q in range(4):
                        sp = q if q < 3 else 32
                        nc.sync.dma_start(
                            tq[q][c * ppc : (c + 1) * ppc, :], stage[sp : sp + 1, :]
                        )

            # ================= phase 3: finalize =================
            fin = hpool.tile([128, 6 * ncols], F32, tag="fin")

            def fcol(i):
                return fin[:, i * ncols : (i + 1) * ncols]

            nc.vector.tensor_tensor(fcol(0), tq[1][:], tq[2][:], ALU.mult)  # n1*n2
            nc.vector.tensor_scalar(fcol(2), fcol(0), 1e-30, None, ALU.add)
            nc.scalar.activation(fcol(1), fcol(2), AF.Sqrt)
            nc.vector.reciprocal(fcol(2), fcol(1))
            nc.vector.tensor_tensor(fcol(0), tq[0][:], fcol(2), ALU.mult)  # s_math
            nc.vector.tensor_scalar(fcol(1), fcol(0), 0.0, 1.0, ALU.max, ALU.min)
            nc.scalar.activation(fcol(3), tq[3][:], AF.Sigmoid, bias=b4s[:, 0:1])
            nc.vector.tensor_scalar(fcol(4), fcol(1), alphas[:, 0:1], None, ALU.mult)
            nc.vector.tensor_scalar(fcol(5), fcol(3), betas[:, 0:1], None, ALU.mult)
            nc.vector.tensor_tensor(fcol(0), fcol(4), fcol(5), ALU.add)
            nc.vector.tensor_scalar(fcol(1), fcol(0), 0.0, 1.0, ALU.max, ALU.min)
            nc.sync.dma_start(
                out_d.ap().rearrange("(p k) -> p k", p=128), fcol(1)
            )

    nc.compile()
    return nc


_NC_CACHE: dict = {}


def _get_nc(n_chunks):
    if n_chunks not in _NC_CACHE:
        _NC_CACHE[n_chunks] = build_nc(n_chunks)
    return _NC_CACHE[n_chunks]


def _prep_weights(W1, gamma, beta_bn, W2, b2, W3, b3, W4, b4, alpha, beta):
    f16 = np.float16
    f32 = np.float32
    W1 = np.asarray(W1, f32)
    W3 = np.asarray(W3, f32)
    w3e = np.zeros((5, 128, 67), f32)
    w3e[0, :, 0:64] = W3[0:128]
    w3e[1, :, 0:64] = W3[128:256]
    w3e[2, :, 0:64] = W3[256:384]
    w3e[0, :, 64] = 1.0  # dot = ones . (h1*h2)
    w3e[3, :, 65] = 1.0  # n1  = ones . h1^2
    w3e[4, :, 66] = 1.0  # n2  = ones . h2^2
    return {
        "w1p": np.ascontiguousarray(W1.reshape(4, 128, H).astype(f16)),
        "w2p": np.ascontiguousarray(np.asarray(W2, f32).reshape(2, 128, 128).astype(f16)),
        "w3e": w3e.astype(f16),
        "w4p": np.asarray(W4, f32).reshape(64, 1).astype(f16),
        "b2c": np.asarray(b2, f32).reshape(128, 1),
        "b3c": np.asarray(b3, f32).reshape(64, 1),
        "b4c": np.full((128, 1), np.asarray(b4, f32).reshape(-1)[0], f32),
        "gamma2": np.ascontiguousarray(np.asarray(gamma, f32).reshape(2, 128).T),
        "betabn2": np.ascontiguousarray(np.asarray(beta_bn, f32).reshape(2, 128).T),
        "alphab": np.full((128, 1), np.asarray(alpha, f32).reshape(-1)[0], f32),
        "betab": np.full((128, 1), np.asarray(beta, f32).reshape(-1)[0], f32),
        "iden": np.eye(128, dtype=f16),
        "b2r": np.asarray(b2, f32).reshape(1, 128).astype(f16),
        "b3r": np.concatenate([np.asarray(b3, f32).reshape(1, 64),
                               np.zeros((1, 3), f32)], axis=1).astype(f16),
    }


def run_on_hw(x1, x2, weights, n_chunks, trace=False):
    R = n_chunks * 512
    nc = _get_nc(n_chunks)
    in_maps = []
    for c in range(N_CORES):
        m = {"x1": np.ascontiguousarray(x1[c * R : (c + 1) * R]),
             "x2": np.ascontiguousarray(x2[c * R : (c + 1) * R])}
        m.update(weights)
        in_maps.append(m)
    r = bass_utils.run_bass_kernel_spmd(
        nc, in_maps, core_ids=list(range(N_CORES)), trace=trace
    )
    out = np.concatenate([r.results[c]["out"] for c in range(N_CORES)])
    return out, r


def kernel(x1, x2, W1, b1, gamma, beta_bn, W2, b2, W3, b3, W4, b4, alpha, beta):
    x1 = np.asarray(x1, np.float32)
    x2 = np.asarray(x2, np.float32)
    n_chunks = x1.shape[0] // (N_CORES * 512)
    weights = _prep_weights(W1, gamma, beta_bn, W2, b2, W3, b3, W4, b4, alpha, beta)
    out, _ = run_on_hw(x1, x2, weights, n_chunks)
    return out.astype(np.float32)


# revision 37
# speedup vs baseline: 1.1370x; 1.1370x over previous
"""BilinearRelationNet Trainium2 kernel (8 NeuronCores, data-parallel over batch).

Layout strategy: feature-on-partitions, batch-on-free-dim ("feature-major").
  phase 1: stream x (fp32) -> cast fp16 -> xbar-transpose -> h_pre = x@W1 in PSUM
           (fp32), bn_stats per chunk, store h_pre as fp16 in SBUF.
  AllReduce: per-tower/per-h-chunk (mean, E[h^2]) across the 8 cores (4 KB).
  phase 2: BN affine+relu -> @W2+relu -> elementwise combine -> @W3ext
           (W3 extended with ones columns so dot/n1/n2 row-sums ride along in
           the same PSUM tile) -> relu -> @W4 -> stage per-row scalars.
  phase 3: gather per-row scalars into [128, R/128] tiles, cosine+sigmoid
           finalization, DMA out.

b1 is dropped: BatchNorm subtracts the batch mean of (x@W1 + b1), so b1
cancels exactly.
"""

import sys

sys.path.insert(0, "/opt/trn_rl_repo")

import numpy as np
import concourse.bass as bass
import concourse.bacc as bacc
import concourse.tile as tile
import concourse.mybir as mybir
from concourse import bass_utils

F32 = mybir.dt.float32
F16 = mybir.dt.float16
AF = mybir.ActivationFunctionType
ALU = mybir.AluOpType

N_CORES = 8
D = 512
H = 256
BN_EPS = 1e-5


def build_nc(n_chunks: int):
    """One SPMD program; each core handles R = n_chunks*512 rows of both x1/x2."""
    R = n_chunks * 512
    nc = bacc.Bacc("TRN2", target_bir_lowering=False, debug=False, num_devices=N_CORES)

    x_dram = [
        nc.dram_tensor("x1", [R, D], F32, kind="ExternalInput"),
        nc.dram_tensor("x2", [R, D], F32, kind="ExternalInput"),
    ]
    w1_d = nc.dram_tensor("w1p", [4, 128, H], F16, kind="ExternalInput")
    w2_d = nc.dram_tensor("w2p", [2, 128, 128], F16, kind="ExternalInput")
    w3_d = nc.dram_tensor("w3e", [5, 128, 67], F16, kind="ExternalInput")
    w4_d = nc.dram_tensor("w4p", [64, 1], F16, kind="ExternalInput")
    b2_d = nc.dram_tensor("b2c", [128, 1], F32, kind="ExternalInput")
    b3_d = nc.dram_tensor("b3c", [64, 1], F32, kind="ExternalInput")
    b4_d = nc.dram_tensor("b4c", [128, 1], F32, kind="ExternalInput")
    gamma_d = nc.dram_tensor("gamma2", [128, 2], F32, kind="ExternalInput")
    betabn_d = nc.dram_tensor("betabn2", [128, 2], F32, kind="ExternalInput")
    alpha_d = nc.dram_tensor("alphab", [128, 1], F32, kind="ExternalInput")
    beta_d = nc.dram_tensor("betab", [128, 1], F32, kind="ExternalInput")
    iden_d = nc.dram_tensor("iden", [128, 128], F16, kind="ExternalInput")
    b2r_d = nc.dram_tensor("b2r", [1, 128], F16, kind="ExternalInput")
    b3r_d = nc.dram_tensor("b3r", [1, 67], F16, kind="ExternalInput")
    out_d = nc.dram_tensor("out", [R], F32, kind="ExternalOutput")

    with tile.TileContext(nc) as tc:
        with (
            tc.tile_pool(name="const", bufs=1) as cpool,
            tc.tile_pool(name="persist", bufs=1) as hpool,
            tc.tile_pool(name="dram", bufs=1, space="DRAM") as dpool,
        ):
            # ---- constants to SBUF ----
            w1s = cpool.tile([128, 4 * H], F16, tag="w1s")
            for dc in range(4):
                nc.sync.dma_start(w1s[:, dc * H : (dc + 1) * H], w1_d[dc])
            w2s = cpool.tile([128, 2 * 128], F16, tag="w2s")
            for k in range(2):
                nc.sync.dma_start(w2s[:, k * 128 : (k + 1) * 128], w2_d[k])
            w3s = cpool.tile([128, 5 * 67], F16, tag="w3s")
            for k in range(5):
                nc.sync.dma_start(w3s[:, k * 67 : (k + 1) * 67], w3_d[k])
            w4s = cpool.tile([64, 1], F16, tag="w4s")
            nc.sync.dma_start(w4s[:], w4_d[:])
            b2s = cpool.tile([128, 1], F32, tag="b2s")
            nc.sync.dma_start(b2s[:], b2_d[:])
            b3s = cpool.tile([64, 1], F32, tag="b3s")
            nc.sync.dma_start(b3s[:], b3_d[:])
            b4s = cpool.tile([128, 1], F32, tag="b4s")
            nc.sync.dma_start(b4s[:], b4_d[:])
            gammas = cpool.tile([128, 2], F32, tag="gammas")
            nc.sync.dma_start(gammas[:], gamma_d[:])
            betabns = cpool.tile([128, 2], F32, tag="betabns")
            nc.sync.dma_start(betabns[:], betabn_d[:])
            alphas = cpool.tile([128, 1], F32, tag="alphas")
            nc.sync.dma_start(alphas[:], alpha_d[:])
            betas = cpool.tile([128, 1], F32, tag="betas")
            nc.sync.dma_start(betas[:], beta_d[:])
            idens = cpool.tile([128, 128], F16, tag="idens")
            nc.sync.dma_start(idens[:], iden_d[:])
            b2rs = cpool.tile([1, 128], F16, tag="b2rs")
            nc.sync.dma_start(b2rs[:], b2r_d[:])
            b3rs = cpool.tile([1, 67], F16, tag="b3rs")
            nc.sync.dma_start(b3rs[:], b3r_d[:])
            ones_row = cpool.tile([1, 512], F16, tag="ones_row")
            nc.vector.memset(ones_row[:], 1.0)

            # ---- persistent buffers ----
            # h_pre fp16, per (tower, h-chunk): [128, R]
            hp = [
                [hpool.tile([128, R], F16, tag=f"hp{t}{m}", name=f"hp{t}{m}") for m in range(2)]
                for t in range(2)
            ]
            # bn_stats accumulators per stat-set (t,m): [128, 6*n_chunks]
            sbstats = [
                hpool.tile([128, 6 * n_chunks], F32, tag=f"bst{s}", name=f"bst{s}") for s in range(4)
            ]
            # per-row scalar gather targets: dot, n1, n2, slearn
            ncols = R // 128
            tq = [hpool.tile([128, ncols], F32, tag=f"tq{q}", name=f"tq{q}") for q in range(4)]
            # BN scale/shift per stat-set
            gsh = hpool.tile([128, 8], F32, tag="gsh")

            # ================= phase 1 =================
            # fp32->fp16 cast via SWDGE casting DMA (DRAM->SBUF), transpose on
            # TensorE (fp16, PSUM out), evacuate PSUM->SBUF in [128,512] strips.
            # Towers are processed sequentially so each tower's stats AllReduce
            # overlaps with the next tower's / phase 2's compute.
            stw = hpool.tile([128, 16], F32, tag="stw")  # scratch
            st2 = hpool.tile([128, 16], F32, tag="st2")  # scratch 2
            arin = hpool.tile([128, 8], F32, tag="arin")
            arout = hpool.tile([128, 8], F32, tag="arout")
            with (
                tc.tile_pool(name="p1sb", bufs=3) as p1,
                tc.tile_pool(name="p1ps", bufs=2, space="PSUM") as pp1,
            ):
                for t in range(2):
                    for c in range(n_chunks):
                        rows = slice(c * 512, (c + 1) * 512)
                        xfs = p1.tile([128, 4 * 512], F16, tag="xfs")
                        nc.gpsimd.dma_start(
                            xfs.rearrange("p (nb d) -> p nb d", nb=4),
                            x_dram[t][rows, :].rearrange("(nb p) d -> p nb d", p=128),
                        )
                        xT = p1.tile([128, 4 * 512], F16, tag="xT")
                        for half in range(2):
                            # one fp16 PSUM bank holds 8 transposed blocks
                            ptr = pp1.tile([128, 1024], F16, tag="ptr", bufs=3)
                            for dci in range(2):
                                dc = half * 2 + dci
                                for nb in range(4):
                                    nc.tensor.transpose(
                                        ptr[:, dci * 512 + nb * 128 : dci * 512 + (nb + 1) * 128],
                                        xfs[:, nb * 512 + dc * 128 : nb * 512 + (dc + 1) * 128],
                                        idens[:],
                                    )
                            if half == 0:
                                nc.vector.tensor_copy(
                                    xT[:, half * 1024 : (half + 1) * 1024], ptr[:]
                                )
                            else:
                                nc.scalar.activation(
                                    xT[:, half * 1024 : (half + 1) * 1024], ptr[:],
                                    AF.Copy,
                                )
                        for m in range(2):
                            ps = pp1.tile([128, 512], F32, tag=f"ps{m}")
                            for dc in range(4):
                                nc.tensor.matmul(
                                    ps[:],
                                    w1s[:, dc * H + m * 128 : dc * H + (m + 1) * 128],
                                    xT[:, dc * 512 : (dc + 1) * 512],
                                    start=(dc == 0),
                                    stop=(dc == 3),
                                )
                            s = t * 2 + m
                            nc.vector.bn_stats(
                                sbstats[s][:, c * 6 : (c + 1) * 6], ps[:]
                            )
                            nc.scalar.activation(
                                hp[t][m][:, c * 512 : (c + 1) * 512], ps[:], AF.Copy
                            )

                    # ---- tower-t stats + AllReduce (overlaps later work) ----
                    for m in range(2):
                        s = t * 2 + m
                        aggr = stw[:, s * 2 : s * 2 + 2]
                        nc.vector.bn_aggr(aggr, sbstats[s][:])
                        # arin[2m] = mean ; arin[2m+1] = E[h^2] = var + mean^2
                        nc.vector.tensor_copy(
                            arin[:, 4 * t + 2 * m : 4 * t + 2 * m + 1], aggr[:, 0:1]
                        )
                        msq = stw[:, 8 + s : 9 + s]
                        nc.vector.tensor_tensor(msq, aggr[:, 0:1], aggr[:, 0:1], ALU.mult)
                        nc.vector.tensor_tensor(
                            arin[:, 4 * t + 2 * m + 1 : 4 * t + 2 * m + 2],
                            aggr[:, 1:2], msq, ALU.add,
                        )
                # The Tile scheduler effectively barriers the chip around each
                # collective, so use ONE small AllGather (lower floor than
                # AllReduce) and do the 8-way sum locally on DVE.
                bnc_in = dpool.tile([128, 8], F32, name="bnc_in")
                bnc_out = dpool.tile(
                    [128 * N_CORES, 8], F32, addr_space="Shared", name="bnc_out"
                )
                nc.sync.dma_start(bnc_in[:], arin[:])
                nc.gpsimd.collective_compute(
                    "AllGather",
                    ALU.bypass,
                    ins=[bnc_in.opt()],
                    outs=[bnc_out.opt()],
                    replica_groups=[list(range(N_CORES))],
                )
                # gather all ranks' stats into [128, 8 ranks * 8 cols]
                agout = hpool.tile([128, 64], F32, tag="agout")
                nc.sync.dma_start(
                    agout.rearrange("p (r j) -> p r j", r=N_CORES),
                    bnc_out.rearrange("(r p) j -> p r j", p=128),
                )
                # tree-sum the 8 rank blocks -> arout [128, 8]
                tsum = hpool.tile([128, 32], F32, tag="tsum")
                for i in range(4):
                    nc.vector.tensor_tensor(
                        tsum[:, 8 * i : 8 * i + 8],
                        agout[:, 16 * i : 16 * i + 8],
                        agout[:, 16 * i + 8 : 16 * i + 16],
                        ALU.add,
                    )
                for i in range(2):
                    nc.vector.tensor_tensor(
                        tsum[:, 8 * i : 8 * i + 8],
                        tsum[:, 16 * i : 16 * i + 8],
                        tsum[:, 16 * i + 8 : 16 * i + 16],
                        ALU.add,
                    )
                nc.vector.tensor_tensor(
                    arout[:], tsum[:, 0:8], tsum[:, 8:16], ALU.add
                )

                # vectorized across the 4 stat-sets via strided column APs:
                # arout cols [0,2,4,6]=sum(mean_l), [1,3,5,7]=sum(Eh2_l)
                gm4 = st2[:, 0:4]
                nc.vector.tensor_scalar(
                    gm4, arout[:, 0:8:2], 1.0 / N_CORES, None, ALU.mult
                )
                ge4 = st2[:, 4:8]
                nc.vector.tensor_scalar(
                    ge4, arout[:, 1:8:2], 1.0 / N_CORES, None, ALU.mult
                )
                gm2_4 = st2[:, 8:12]
                nc.vector.tensor_tensor(gm2_4, gm4, gm4, ALU.mult)
                var4 = st2[:, 12:16]
                nc.vector.tensor_tensor(var4, ge4, gm2_4, ALU.subtract)
                vare4 = stw[:, 0:4]
                nc.vector.tensor_scalar(vare4, var4, float(BN_EPS), None, ALU.add)
                std4 = stw[:, 4:8]
                nc.scalar.activation(std4, vare4, AF.Sqrt)
                istd4 = stw[:, 8:12]
                nc.vector.reciprocal(istd4, std4)
                # gamma/beta col m for set s=(t,m): order gamma cols [0,1,0,1]
                gam4 = stw[:, 12:16]
                nc.vector.tensor_copy(gam4[:, 0:2], gammas[:])
                nc.vector.tensor_copy(gam4[:, 2:4], gammas[:])
                bet4 = st2[:, 0:4]  # gm4 no longer needed after gmg
                gmg4 = st2[:, 4:8]  # reuse ge4 slot after use
                nc.vector.tensor_tensor(
                    gsh[:, 0:8:2], istd4, gam4, ALU.mult
                )
                nc.vector.tensor_tensor(gmg4, gm4, gsh[:, 0:8:2], ALU.mult)
                nc.vector.tensor_copy(bet4[:, 0:2], betabns[:])
                nc.vector.tensor_copy(bet4[:, 2:4], betabns[:])
                nc.vector.tensor_tensor(
                    gsh[:, 1:8:2], bet4, gmg4, ALU.subtract
                )

            # ================= phase 2 =================
            # Process CW=1024 column blocks (2 chunks) to amortize the fixed
            # per-op engine overheads (ACT ~224cyc/op). Matmuls still target
            # one PSUM bank (N=512) per instruction.
            CW = 1024 if n_chunks % 2 == 0 else 512
            NH = CW // 512  # matmul halves per block
            with (
                tc.tile_pool(name="p2sb", bufs=3) as p2,
                tc.tile_pool(name="p2ps", bufs=1, space="PSUM") as pp2,
            ):
                for c in range(R // CW):
                    cols = slice(c * CW, (c + 1) * CW)
                    h = []
                    for t in range(2):
                        hn = []
                        for m in range(2):
                            s = t * 2 + m
                            src = hp[t][m][:, cols]
                            dst = p2.tile([128, CW], F16, tag=f"hn{t}{m}")
                            if s < 2:
                                nc.scalar.activation(
                                    dst, src, AF.Relu,
                                    scale=gsh[:, 2 * s : 2 * s + 1],
                                    bias=gsh[:, 2 * s + 1 : 2 * s + 2],
                                )
                            else:
                                tmp = p2.tile([128, CW], F16, tag=f"hntmp{t}{m}")
                                nc.vector.tensor_scalar(
                                    tmp, src,
                                    gsh[:, 2 * s : 2 * s + 1],
                                    gsh[:, 2 * s + 1 : 2 * s + 2],
                                    ALU.mult, ALU.add,
                                )
                                nc.vector.tensor_scalar(
                                    dst, tmp, 0.0, None, ALU.max
                                )
                            hn.append(dst)
                        pw = pp2.tile([128, CW], F32, tag=f"pw{t}", bufs=1)
                        for hf in range(NH):
                            hs = slice(hf * 512, (hf + 1) * 512)
                            nc.tensor.matmul(
                                pw[:, hs], w2s[:, 0:128], hn[0][:, hs],
                                start=True, stop=False,
                            )
                            nc.tensor.matmul(
                                pw[:, hs], w2s[:, 128:256], hn[1][:, hs],
                                start=False, stop=False,
                            )
                            nc.tensor.matmul(
                                pw[:, hs], b2rs[:], ones_row[:],
                                start=False, stop=True,
                            )
                        ht = p2.tile([128, CW], F16, tag=f"h{t}")
                        if t == 0:
                            nc.scalar.activation(ht, pw[:], AF.Relu)
                        else:
                            nc.vector.tensor_scalar(ht, pw[:], 0.0, None, ALU.max)
                        h.append(ht)

                    p_t = p2.tile([128, CW], F16, tag="p_t")
                    nc.vector.tensor_tensor(p_t[:], h[0][:], h[1][:], ALU.mult)
                    dd = p2.tile([128, CW], F16, tag="dd")
                    nc.vector.tensor_tensor(dd[:], h[0][:], h[1][:], ALU.subtract)
                    q_t = p2.tile([128, CW], F16, tag="q_t")
                    nc.scalar.activation(q_t[:], dd[:], AF.Abs)
                    r_t = p2.tile([128, CW], F16, tag="r_t")
                    nc.vector.tensor_tensor(r_t[:], h[0][:], h[1][:], ALU.add)
                    s1_t = p2.tile([128, CW], F16, tag="s1_t")
                    nc.scalar.activation(s1_t[:], h[0][:], AF.Square)
                    s2_t = p2.tile([128, CW], F16, tag="s2_t")
                    nc.gpsimd.tensor_tensor(s2_t[:], h[1][:], h[1][:], ALU.mult)

                    pw3 = pp2.tile([128, CW], F32, tag="pw3", bufs=1)
                    rhs5 = [p_t, q_t, r_t, s1_t, s2_t]
                    for hf in range(NH):
                        hs = slice(hf * 512, (hf + 1) * 512)
                        for k in range(5):
                            nc.tensor.matmul(
                                pw3[0:67, hs],
                                w3s[:, k * 67 : (k + 1) * 67],
                                rhs5[k][:, hs],
                                start=(k == 0),
                                stop=False,
                            )
                        nc.tensor.matmul(
                            pw3[0:67, hs], b3rs[:], ones_row[:],
                            start=False, stop=True,
                        )
                    r64 = p2.tile([64, CW], F16, tag="r64")
                    nc.vector.tensor_scalar(r64[:], pw3[0:64, :], 0.0, None, ALU.max)
                    stage = p2.tile([33, CW], F32, tag="stage")
                    nc.scalar.activation(stage[0:3, :], pw3[64:67, :], AF.Copy)
                    pw4 = pp2.tile([1, CW], F32, tag="pw4", bufs=1)
                    for hf in range(NH):
                        hs = slice(hf * 512, (hf + 1) * 512)
                        nc.tensor.matmul(
                            pw4[:, hs], w4s[:], r64[:, hs], start=True, stop=True
                        )
                    nc.vector.tensor_copy(stage[32:33, :], pw4[:])
                    ppc = CW // ncols  # partition rows of tq covered per block
                    for q in range(4):
                        sp = q if q < 3 else 32
                        nc.sync.dma_start(
                            tq[q][c * ppc : (c + 1) * ppc, :], stage[sp : sp + 1, :]
                        )

            # ================= phase 3: finalize =================
            fin = hpool.tile([128, 6 * ncols], F32, tag="fin")

            def fcol(i):
                return fin[:, i * ncols : (i + 1) * ncols]

            nc.vector.tensor_tensor(fcol(0), tq[1][:], tq[2][:], ALU.mult)  # n1*n2
            nc.vector.tensor_scalar(fcol(2), fcol(0), 1e-30, None, ALU.add)
            nc.scalar.activation(fcol(1), fcol(2), AF.Sqrt)
            nc.vector.reciprocal(fcol(2), fcol(1))
            nc.vector.tensor_tensor(fcol(0), tq[0][:], fcol(2), ALU.mult)  # s_math
            nc.vector.tensor_scalar(fcol(1), fcol(0), 0.0, 1.0, ALU.max, ALU.min)
            nc.scalar.activation(fcol(3), tq[3][:], AF.Sigmoid, bias=b4s[:, 0:1])
            nc.vector.tensor_scalar(fcol(4), fcol(1), alphas[:, 0:1], None, ALU.mult)
            nc.vector.tensor_scalar(fcol(5), fcol(3), betas[:, 0:1], None, ALU.mult)
            nc.vector.tensor_tensor(fcol(0), fcol(4), fcol(5), ALU.add)
            nc.vector.tensor_scalar(fcol(1), fcol(0), 0.0, 1.0, ALU.max, ALU.min)
            nc.sync.dma_start(
                out_d.ap().rearrange("(p k) -> p k", p=128), fcol(1)
            )

    nc.compile()
    return nc


_NC_CACHE: dict = {}


def _get_nc(n_chunks):
    if n_chunks not in _NC_CACHE:
        _NC_CACHE[n_chunks] = build_nc(n_chunks)
    return _NC_CACHE[n_chunks]


def _prep_weights(W1, gamma, beta_bn, W2, b2, W3, b3, W4, b4, alpha, beta):
    f16 = np.float16
    f32 = np.float32
    W1 = np.asarray(W1, f32)
    W3 = np.asarray(W3, f32)
    w3e = np.zeros((5, 128, 67), f32)
    w3e[0, :, 0:64] = W3[0:128]
    w3e[1, :, 0:64] = W3[128:256]
    w3e[2, :, 0:64] = W3[256:384]
    w3e[0, :, 64] = 1.0  # dot = ones . (h1*h2)
    w3e[3, :, 65] = 1.0  # n1  = ones . h1^2
    w3e[4, :, 66] = 1.0  # n2  = ones . h2^2
    return {
        "w1p": np.ascontiguousarray(W1.reshape(4, 128, H).astype(f16)),
        "w2p": np.ascontiguousarray(np.asarray(W2, f32).reshape(2, 128, 128).astype(f16)),
        "w3e": w3e.astype(f16),
        "w4p": np.asarray(W4, f32).reshape(64, 1).astype(f16),
        "b2c": np.asarray(b2, f32).reshape(128, 1),
        "b3c": np.asarray(b3, f32).reshape(64, 1),
        "b4c": np.full((128, 1), np.asarray(b4, f32).reshape(-1)[0], f32),
        "gamma2": np.ascontiguousarray(np.asarray(gamma, f32).reshape(2, 128).T),
        "betabn2": np.ascontiguousarray(np.asarray(beta_bn, f32).reshape(2, 128).T),
        "alphab": np.full((128, 1), np.asarray(alpha, f32).reshape(-1)[0], f32),
        "betab": np.full((128, 1), np.asarray(beta, f32).reshape(-1)[0], f32),
        "iden": np.eye(128, dtype=f16),
        "b2r": np.asarray(b2, f32).reshape(1, 128).astype(f16),
        "b3r": np.concatenate([np.asarray(b3, f32).reshape(1, 64),
                               np.zeros((1, 3), f32)], axis=1).astype(f16),
    }


def run_on_hw(x1, x2, weights, n_chunks, trace=False):
    R = n_chunks * 512
    nc = _get_nc(n_chunks)
    in_maps = []
    for c in range(N_CORES):
        m = {"x1": np.ascontiguousarray(x1[c * R : (c + 1) * R]),
             "x2": np.ascontiguousarray(x2[c * R : (c + 1) * R])}
        m.update(weights)
        in_maps.append(m)
    r = bass_utils.run_bass_kernel_spmd(
        nc, in_maps, core_ids=list(range(N_CORES)), trace=trace
    )
    out = np.concatenate([r.results[c]["out"] for c in range(N_CORES)])
    return out, r


def kernel(x1, x2, W1, b1, gamma, beta_bn, W2, b2, W3, b3, W4, b4, alpha, beta):
    x1 = np.asarray(x1, np.float32)
    x2 = np.asarray(x2, np.float32)
    n_chunks = x1.shape[0] // (N_CORES * 512)
    weights = _prep_weights(W1, gamma, beta_bn, W2, b2, W3, b3, W4, b4, alpha, beta)
    out, _ = run_on_hw(x1, x2, weights, n_chunks)
    return out.astype(np.float32)
